# revision 1
# baseline (speedup 1.0000x reference)
"""Causal self-attention (dense transformer attn layer) on 8 Trainium2
NeuronCores.

Sharding: batch x head-group.  Core c handles batch b = c//2 and head-group
g = c%2 (8 of 16 heads).  Each core computes the qkv projection for its head
slice (column-parallel), full causal attention for its 8 heads, and a
row-parallel slice of the output projection.  The host sums the two partial
projection outputs per batch (the "all-reduce") and adds b_proj plus the
v-bias contribution (bv @ W_proj — exact because softmax rows sum to 1).

On-core DRAM layout (per core, T=2048, C=1024, HLOC=8 heads, D=64):
  xT   [C, T]    input slice, transposed on host      (bf16)
  wqk  [C, 1024] W_attn columns for q (512) + k (512) (bf16)
  wv   [C, 512]  W_attn columns for v                 (bf16)
  wpr  [512, C]  W_proj rows for this group           (bf16)
  bqk  [128, 8]  q/k bias per dout-chunk column       (f32)
  out  [T, C]    partial output                       (f32)

All big matmuls run in bf16 with fp32 PSUM accumulation.  Weight and x
loads are single rearranged DMAs (HWDGE issue is ~0.6us per dma_start, so
count dominates, not bytes); the k/v/proj weights ride the Activation
HWDGE ring in parallel with the Sync ring.

Pipeline: per 512-token i-chunk c, the attention of chunk c is interleaved
with the qkv projection of chunk c+1 so the PE stays dense.  ALL output
projections are deferred to the final chunk's attention phase, which is
exp-throughput-bound on the Scalar engine and has spare PE capacity.
Per i-chunk:
  1. q(i)^T, k(i)^T = w-stationary @ x(i)^T          -> [d, t] layout
  2. v(i)   = x(i)^T-stationary @ wv -> [t, d] + ones column (for l)
  3. per head-pair, per causal j-block (128 keys):
       S^T[j, i] = k^T(j)-stationary @ q^T(i)   (d=64 row-pair packed, the
                                                 two heads' MMs run
                                                 concurrently in the PE)
       P^T       = exp(S^T/sqrt(d)); diagonal blocks masked via a
                   multiplicative bf16 mask and column-trimmed (both the
                   S and the PV matmuls skip the all-zero column range)
       Yaug^T   += [V|1](j)-stationary @ P^T     (contract j=128, accum)
     Yaug^T row 64 is the softmax denominator l(i).
  4. normalize (DEFERRED into the next head-pair's attention as filler):
     evacuate Yaug^T to SBUF, gather the two l rows into a [2, IC] tile
     (SBUF-SBUF DMA partition shift), 1/l via the DVE fast reciprocal
     (keeps the Scalar engine free for the attention exps), broadcast 1/l
     to 64 partitions via a DRAM bounce, multiply.  The chain DMAs ride
     the GpSimd SWDGE ring so bulk traffic on the Sync ring can't delay
     them (except the final chunk's last pair, which uses Sync for
     latency and is issued ahead of the tail's store traffic).
  5. out(i) = y^T-stationary @ wpr  (contract d=512): chunks 0-2 all run
     as filler inside the final chunk's attention; chunk 2's set is held
     back to keep the PE busy under the tail normalization chain.  The
     final chunk's own projection runs per-head-pair (PSUM-free SBUF
     accumulation) interleaved with its own attention.
"""

import numpy as np

# ---------------------------------------------------------------- constants
B, T, C = 4, 2048, 1024
H, D = 16, 64
NCORES = 8
HGROUPS = NCORES // B          # 2 head groups
HLOC = H // HGROUPS            # 8 heads per core
DQ = HLOC * D                  # 512 head-dims per core
P = 128
IC = 512                       # i-chunk (query) width


def _import_concourse():
    try:
        import concourse.bass  # noqa: F401
    except ImportError:
        import sys

        for p in ("/opt/trn_rl_repo", "/root/.axon_site/_ro/trn_rl_repo"):
            if p not in sys.path:
                sys.path.insert(0, p)
        import concourse.bass  # noqa: F401


def build_program(t=T, c=C, hloc=HLOC, d=D):
    """Build the single-core Bass program (the same program runs SPMD on 8)."""
    _import_concourse()
    import concourse.bass as bass
    import concourse.mybir as mybir
    import concourse.tile as tile

    assert c % P == 0 and t % IC == 0 and hloc % 2 == 0 and d == 64
    dq = hloc * d                  # local q/k/v width
    CK = c // P                    # contraction chunks over channels
    TI = t // IC                   # i-chunks
    JPC = IC // P                  # j-blocks per i-chunk (4)
    DCH = dq // P                  # q/k/y dout chunks
    HP = hloc // 2                 # head pairs
    F32 = mybir.dt.float32
    BF16 = mybir.dt.bfloat16
    EXP = mybir.ActivationFunctionType.Exp
    LN = mybir.ActivationFunctionType.Ln
    SCALE = 1.0 / float(np.sqrt(d))

    # all big inputs are pre-shuffled on the host into partition-major
    # layouts so every load is a fully contiguous 2D DMA on both sides
    # (strided patterns degrade to 1KB packets at ~90ns each on the SDMA
    # engines — ~4x below line rate)
    nc = bass.Bass()
    xT = nc.declare_dram_parameter("xT", [P, TI * CK * IC], BF16,
                                   isOutput=False)
    wq = nc.declare_dram_parameter("wq", [P, CK * dq], BF16, isOutput=False)
    wk = nc.declare_dram_parameter("wk", [P, CK * dq], BF16, isOutput=False)
    wv = nc.declare_dram_parameter("wv", [P, CK * dq], BF16, isOutput=False)
    wpr = nc.declare_dram_parameter("wpr", [P, DCH * c], BF16, isOutput=False)
    bqk = nc.declare_dram_parameter("bqk", [P, 2 * DCH], F32, isOutput=False)
    out = nc.declare_dram_parameter("out", [t, c], F32, isOutput=True)

    with tile.TileContext(nc) as tc:
        with (
            nc.allow_low_precision(reason="bf16 matmul inputs, fp32 accum"),
            tc.tile_pool(name="const", bufs=1) as const,
            tc.tile_pool(name="xin", bufs=2) as xin,
            tc.tile_pool(name="qpool", bufs=2) as qpool,
            tc.tile_pool(name="kpool", bufs=TI) as kpool,
            tc.tile_pool(name="vpool", bufs=TI) as vpool,
            tc.tile_pool(name="ypool", bufs=TI) as ypool,
            tc.tile_pool(name="ptp", bufs=10) as ptp,
            tc.tile_pool(name="bcp", bufs=2) as bcp,
            tc.tile_pool(name="ytp", bufs=2) as ytp,
            tc.tile_pool(name="ostage", bufs=2) as ostage,
            tc.tile_pool(name="oacc", bufs=1) as oaccp,
            tc.tile_pool(name="drp", bufs=4, space="DRAM") as drp,
            tc.tile_pool(name="ps_mm", bufs=2, space="PSUM") as ps_mm,
            tc.tile_pool(name="ps_st", bufs=2, space="PSUM") as ps_st,
            tc.tile_pool(name="ps_y", bufs=2, space="PSUM") as ps_y,
        ):
            # ---------------- persistent SBUF state
            wq_sb = const.tile([P, CK, dq], BF16)
            wk_sb = const.tile([P, CK, dq], BF16)
            wv_sb = const.tile([P, CK, dq], BF16)
            wpr_sb = const.tile([P, DCH, c], BF16)
            mask_sb = const.tile([P, JPC, IC], BF16)
            ones_bf = const.tile([P, P], BF16)
            bqk_sb = const.tile([P, 2 * DCH], F32)

            # batched weight loads: one contiguous DMA per tensor.  The 16
            # SDMA engines round-robin between the rings at packet
            # granularity, so wk/wv/wpr (not needed until later) are gated
            # behind x0's landing with a dummy Scalar-engine read — else
            # they'd halve the bandwidth available to wq + x0, which gate
            # the first matmul.
            nc.sync.dma_start(out=bqk_sb, in_=bqk[:, :])
            nc.sync.dma_start(out=wq_sb, in_=wq[:, :])

            # memset can't emit bf16 ones via float32r path; fill f32 and
            # round via DVE copy
            ones_f32 = const.tile([P, P], F32)
            nc.vector.memset(ones_f32, 1.0)
            nc.vector.tensor_copy(out=ones_bf, in_=ones_f32)
            # multiplicative causal masks for the 4 diagonal j-block
            # positions: pattern p is 1 where i_local >= j_local + 128*p
            for pat in range(JPC):
                nc.gpsimd.memset(mask_sb[:, pat, :], 1.0)
                nc.gpsimd.affine_select(
                    out=mask_sb[:, pat, :],
                    in_=mask_sb[:, pat, :],
                    compare_op=mybir.AluOpType.is_ge,
                    fill=0.0,
                    base=-(pat * P),
                    pattern=[[1, IC]],
                    channel_multiplier=-1,
                )

            q_tiles = {}
            k_tiles = {}
            v_tiles = {}
            y_tiles = {}

            def load_x(c4):
                xt = xin.tile([P, CK, IC], BF16, tag="x")
                nc.sync.dma_start(
                    out=xt,
                    in_=xT[:, c4 * CK * IC:(c4 + 1) * CK * IC])
                return xt

            def qkv_thunks(c4, xt_pre=None):
                """One thunk per PSUM accumulation group; called interleaved
                with the previous chunk's attention to keep PE dense.  The
                x chunk is loaded lazily when the first group runs, so its
                DMA doesn't compete with earlier traffic."""
                q_cur = qpool.tile([P, DCH, IC], BF16, tag="q")
                k_cur = kpool.tile([P, DCH, IC], BF16, tag="k")
                v_cur = vpool.tile([P, JPC, hloc, d + 1], BF16, tag="v")
                q_tiles[c4] = q_cur
                k_tiles[c4] = k_cur
                v_tiles[c4] = v_cur
                holder = {}
                if xt_pre is not None:
                    holder["xt"] = xt_pre

                def get_xt():
                    if "xt" not in holder:
                        holder["xt"] = load_x(c4)
                    return holder["xt"]

                thunks = []

                def q_group(oc):
                    xt = get_xt()
                    ps = ps_mm.tile([P, 512], F32, tag="mm")
                    for cc in range(CK):
                        nc.tensor.matmul(
                            ps[:, :IC],
                            lhsT=wq_sb[:, cc, oc * P:(oc + 1) * P],
                            rhs=xt[:, cc, :], start=(cc == 0),
                            stop=(cc == CK - 1))
                    nc.vector.tensor_scalar_add(q_cur[:, oc, :], ps[:, :IC],
                                                bqk_sb[:, oc:oc + 1])

                def k_group(oc):
                    xt = get_xt()
                    ps = ps_mm.tile([P, 512], F32, tag="mm")
                    for cc in range(CK):
                        nc.tensor.matmul(
                            ps[:, :IC],
                            lhsT=wk_sb[:, cc, oc * P:(oc + 1) * P],
                            rhs=xt[:, cc, :], start=(cc == 0),
                            stop=(cc == CK - 1))
                    nc.vector.tensor_scalar_add(
                        k_cur[:, oc, :], ps[:, :IC],
                        bqk_sb[:, DCH + oc:DCH + oc + 1])

                def v_group(tbl):
                    xt = get_xt()
                    ps = ps_mm.tile([P, 512], F32, tag="mm")
                    for cc in range(CK):
                        nc.tensor.matmul(
                            ps[:, :dq],
                            lhsT=xt[:, cc, tbl * P:(tbl + 1) * P],
                            rhs=wv_sb[:, cc, :], start=(cc == 0),
                            stop=(cc == CK - 1))
                    nc.vector.tensor_copy(
                        out=v_cur[:, tbl, :, 0:d],
                        in_=ps[:, :dq].rearrange("p (h e) -> p h e", h=hloc))
                    # ones column for the softmax-denominator accumulator
                    nc.vector.tensor_copy(
                        out=v_cur[:, tbl, :, d:d + 1],
                        in_=ones_bf[:, 0:hloc][:, :, None])

                for oc in range(DCH):
                    thunks.append(lambda oc=oc: q_group(oc))
                    thunks.append(lambda oc=oc: k_group(oc))
                for tbl in range(JPC):
                    thunks.append(lambda tbl=tbl: v_group(tbl))
                return thunks

            def attention_hp(c4, hp, filler=()):
                """Attention for head-pair hp of chunk c4.  Returns the two
                un-normalized Yaug^T PSUM tiles (normalization is deferred
                into the next head-pair's filler)."""
                filler = list(filler)
                q_cur = q_tiles[c4]
                njb = (c4 + 1) * JPC
                BLK = 2   # j-blocks per S-burst (matches ps_st bufs)
                nblk = (njb + BLK - 1) // BLK
                fill_every = max(1, nblk // len(filler)) if filler else 0
                ya = ps_y.tile([d + 1, IC], F32, tag="y")
                yb = ps_y.tile([d + 1, IC], F32, tag="y")
                blk_i = 0
                for j0 in range(0, njb, BLK):
                    jbs = range(j0, min(j0 + BLK, njb))
                    # burst of S matmuls + exps, then the PV matmuls — the
                    # exp of tile n hides behind the S matmul of tile n+1,
                    # so the PE never micro-idles
                    pts = {}
                    for jb in jbs:
                        kc, jl = jb // JPC, jb % JPC
                        # both heads' S tiles live in one 2-bank PSUM tile,
                        # so one ACT instruction exps the pair
                        st = ps_st.tile([P, 2, IC], F32, tag="st")
                        pt = ptp.tile([P, 2, IC], BF16, tag="pt")
                        pts[jb] = pt
                        diag = jb >= c4 * JPC
                        pat = jb - c4 * JPC if diag else 0
                        w0 = pat * P if diag else 0
                        for hi, po in ((0, 0), (1, 64)):
                            nc.tensor.matmul(
                                st[:, hi, w0:],
                                lhsT=k_tiles[kc][po:po + 64, hp,
                                                 jl * P:(jl + 1) * P],
                                rhs=q_cur[po:po + 64, hp, w0:],
                                start=True, stop=True)
                        nc.scalar.activation(pt[:, :, w0:], st[:, :, w0:],
                                             EXP, scale=SCALE)
                        if diag:
                            nc.vector.tensor_mul(
                                pt[:, :, w0:w0 + P], pt[:, :, w0:w0 + P],
                                mask_sb[:, pat, None,
                                        w0:w0 + P].to_broadcast(
                                            (P, 2, P)))
                    for jb in jbs:
                        diag = jb >= c4 * JPC
                        w0 = (jb - c4 * JPC) * P if diag else 0
                        for hi, po, yps in ((0, 0, ya), (1, 64, yb)):
                            h = 2 * hp + hi
                            # diagonal blocks only contribute columns >= w0
                            # (everything left of the diagonal strip is
                            # masked to zero), so trim the stream
                            nc.tensor.matmul(
                                yps[:, w0:],
                                lhsT=v_tiles[jb // JPC][:, jb % JPC, h, :],
                                rhs=pts[jb][:, hi, w0:],
                                start=(jb == 0), stop=(jb == njb - 1))
                    blk_i += 1
                    if filler and fill_every and blk_i % fill_every == 0:
                        filler.pop(0)()
                for th in filler:
                    th()
                return ya, yb

            def normalize_thunks(c4, hp, ya, yb, tail=False):
                """Deferred normalization of head-pair hp.  Returns
                (t0, t1, t2): t0 runs inline right after the attention
                (evacuates PSUM so the next pair's PV can start); t1/t2 are
                pushed into the next head-pair's filler, t2 late enough
                that the broadcast DMA has landed.  `tail` (final pair of
                the final chunk) splits the evacuation across DVE+ACT to
                shorten the end-of-kernel chain."""
                y_cur = y_tiles[c4]
                state = {}

                def t0():
                    # evacuate both Yaug^T tiles into one [P, 2, IC] SBUF
                    # staging tile; the PSUM banks free after these copies
                    ycp = bcp.tile([P, 2, IC], F32, tag="ycp")
                    nc.vector.tensor_copy(out=ycp[0:d + 1, 0, :],
                                          in_=ya[0:d + 1, :])
                    if tail:
                        # ACT is idle once the last exp retires; run the
                        # second copy there so the two evacuations overlap
                        nc.scalar.copy(out=ycp[0:d + 1, 1, :],
                                       in_=yb[0:d + 1, :])
                    else:
                        nc.vector.tensor_copy(out=ycp[0:d + 1, 1, :],
                                              in_=yb[0:d + 1, :])
                        # gather the two l rows (partition 64, head slots
                        # 0/1) into partitions 0..1 (the DMA does the
                        # partition shift)
                        lg = bcp.tile([2, IC], F32, tag="lg")
                        nc.sync.dma_start(out=lg[0:2, :],
                                          in_=ycp[d:d + 1, :, :])
                        state["lg"] = lg
                    state["ycp"] = ycp

                def t1():
                    # 1/l = exp(-ln(l)) on ScalarE, batched over both heads
                    # (this walrus build rejects the custom-DVE fast
                    # reciprocal: "ISA wrong length")
                    bcs = bcp.tile([d, 2, IC], F32, tag="bcs")
                    ycp = state["ycp"]
                    if tail:
                        # latency-critical end-of-kernel chain: keep 1/l at
                        # partition 64, broadcast with DVE stream_shuffles
                        # instead of DMA hops, and finish head A entirely
                        # before touching head B so its multiply (which
                        # gates the final projection) starts earliest.
                        lnl = bcp.tile([P, 2, IC], F32, tag="lnl")
                        nc.scalar.activation(lnl[d:d + 1, :, :],
                                             ycp[d:d + 1, :, :], LN)
                        rinv = bcp.tile([P, 2, IC], F32, tag="rinvt")
                        nc.scalar.activation(rinv[d:d + 1, :, :],
                                             lnl[d:d + 1, :, :], EXP,
                                             scale=-1.0)
                        for hi in range(2):
                            nc.vector.stream_shuffle(
                                out=bcs[0:32, hi:hi + 1, :],
                                in_=rinv[64:96, hi:hi + 1, :],
                                mask=[0] * 32)
                            nc.vector.stream_shuffle(
                                out=bcs[32:64, hi:hi + 1, :],
                                in_=rinv[64:96, hi:hi + 1, :],
                                mask=[0] * 32)
                            if hi == 0:
                                nc.vector.tensor_mul(y_cur[0:d, hp, :],
                                                     ycp[0:d, 0, :],
                                                     bcs[:, 0, :])
                    else:
                        lnl = bcp.tile([2, IC], F32, tag="lnl")
                        nc.scalar.activation(lnl, state["lg"], LN)
                        rinv = bcp.tile([2, IC], F32, tag="rinv")
                        nc.scalar.activation(rinv, lnl, EXP, scale=-1.0)
                        rd = drp.tile([2, IC], F32, tag="rd")
                        nc.sync.dma_start(out=rd, in_=rinv)
                        # partition-broadcast 1/l by bouncing through DRAM
                        # (DRAM DMA sources may repeat across partitions;
                        # SBUF sources may not)
                        nc.sync.dma_start(
                            out=bcs,
                            in_=rd[None, :, :].to_broadcast((d, 2, IC)))
                    state["bcs"] = bcs

                def t2():
                    ycp = state["ycp"]
                    bcs = state["bcs"]
                    if not tail:
                        nc.vector.tensor_mul(y_cur[0:d, hp, :],
                                             ycp[0:d, 0, :], bcs[:, 0, :])
                    yt = ytp.tile([P, IC], BF16, tag="yt")
                    nc.vector.tensor_mul(yt[0:d, :],
                                         ycp[0:d, 1, :], bcs[:, 1, :])
                    # shift head B to partitions 64..127
                    if tail:
                        nc.vector.stream_shuffle(
                            out=y_cur[64:96, hp, :], in_=yt[0:32, :],
                            mask=list(range(32)))
                        nc.vector.stream_shuffle(
                            out=y_cur[96:P, hp, :], in_=yt[32:64, :],
                            mask=list(range(32)))
                    else:
                        nc.sync.dma_start(out=y_cur[64:P, hp, :],
                                          in_=yt[0:d, :])

                return t0, t1, t2

            def proj_pair_thunks(c4, pair, oacc):
                """Projection contribution of head-pairs (2*pair, 2*pair+1)
                for the final chunk: two PSUM-accumulated matmuls per output
                tile (denser PE than one-matmul-per-DVE-add), then one DVE
                copy/add into the SBUF accumulator."""
                y_cur = y_tiles[c4]

                def grp(tbl, oh):
                    ps = ps_mm.tile([P, 512], F32, tag="mm")
                    for k, hp in enumerate((2 * pair, 2 * pair + 1)):
                        nc.tensor.matmul(
                            ps,
                            lhsT=y_cur[:, hp, tbl * P:(tbl + 1) * P],
                            rhs=wpr_sb[:, hp, oh * 512:(oh + 1) * 512],
                            start=(k == 0), stop=(k == 1))
                    if pair == 0:
                        nc.vector.tensor_copy(out=oacc[:, tbl, oh, :], in_=ps)
                    else:
                        nc.vector.tensor_add(oacc[:, tbl, oh, :],
                                             oacc[:, tbl, oh, :], ps)
                        tb = c4 * JPC + tbl
                        nc.sync.dma_start(
                            out=out[tb * P:(tb + 1) * P,
                                    oh * 512:(oh + 1) * 512],
                            in_=oacc[:, tbl, oh, :])

                return [lambda tbl=tbl, oh=oh: grp(tbl, oh)
                        for tbl in range(JPC) for oh in range(c // 512)]

            # during the tail the DVE is saturated by the normalization
            # chain; evacuating the held-back projection groups on the
            # (then-idle) Scalar engine keeps ps_mm turning over so the PE
            # never starves behind the DVE queue
            tail_mode = {"on": False}

            def proj_thunks(c4):
                def grp(tbl, oh):
                    y_cur = y_tiles[c4]
                    tb = c4 * JPC + tbl
                    ps = ps_mm.tile([P, 512], F32, tag="mm")
                    for dc in range(DCH):
                        nc.tensor.matmul(
                            ps,
                            lhsT=y_cur[:, dc, tbl * P:(tbl + 1) * P],
                            rhs=wpr_sb[:, dc, oh * 512:(oh + 1) * 512],
                            start=(dc == 0), stop=(dc == DCH - 1))
                    ost = ostage.tile([P, 512], F32, tag="ost")
                    if tail_mode["on"]:
                        nc.scalar.copy(out=ost, in_=ps)
                    else:
                        nc.vector.tensor_copy(out=ost, in_=ps)
                    nc.sync.dma_start(
                        out=out[tb * P:(tb + 1) * P,
                                oh * 512:(oh + 1) * 512],
                        in_=ost)

                return [lambda tbl=tbl, oh=oh: grp(tbl, oh)
                        for tbl in range(JPC) for oh in range(c // 512)]

            # -------------- software pipeline over i-chunks
            xt0 = load_x(0)
            # later-needed weights go on the SAME Sync ring AFTER x0: the
            # ring is FIFO, so wq+x0 (which gate the first matmul) get the
            # full SDMA bandwidth first.  (A second ring halves their
            # bandwidth — the 16 SDMA engines round-robin between rings —
            # and the Tile scheduler reorders cross-engine issue, so a
            # dummy-dependency gate doesn't work.)
            nc.sync.dma_start(out=wk_sb, in_=wk[:, :])
            nc.sync.dma_start(out=wv_sb, in_=wv[:, :])
            nc.sync.dma_start(out=wpr_sb, in_=wpr[:, :])
            for th in qkv_thunks(0, xt_pre=xt0):
                th()
            # projections of chunks 0-2 are ALL deferred to the final
            # chunk's attention phase (exp-bound on ScalarE, spare PE);
            # chunk 2's set is held back as tail filler.
            proj_backlog = []
            norm_head = []
            norm_tail = []
            for c4 in range(TI):
                last = c4 + 1 >= TI
                pend = []
                if not last:
                    pend += qkv_thunks(c4 + 1)
                    oacc = None
                    tail_fill = []
                else:
                    tail_fill = proj_backlog.pop()      # chunk 2's proj set
                    while proj_backlog:
                        pend += proj_backlog.pop(0)     # chunks 0, 1
                    oacc = oaccp.tile([P, JPC, c // 512, 512], F32,
                                      name="oacc")
                y_tiles[c4] = ypool.tile([P, DCH, IC], BF16, tag="ych",
                                         name=f"ych_{c4}")
                per_hp = (len(pend) + HP - 1) // HP if pend else 0
                carry_proj = []
                for hp in range(HP):
                    fill = (norm_head
                            + pend[hp * per_hp:(hp + 1) * per_hp]
                            + norm_tail
                            + carry_proj)
                    carry_proj = []
                    ya, yb = attention_hp(c4, hp, filler=fill)
                    t0, t1, t2 = normalize_thunks(c4, hp, ya, yb,
                                                  tail=(last and hp == HP - 1))
                    t0()
                    norm_head, norm_tail = [t1], [t2]
                    if last and hp == 1:
                        # head-pairs 0+1 project together once both are
                        # normalized (runs in hp2's filler)
                        carry_proj = proj_pair_thunks(c4, 0, oacc)
                if not last:
                    proj_backlog.append(proj_thunks(c4))
                else:
                    # tail: the last head-pair's normalization chain runs
                    # with chunk 2's projection keeping the PE warm
                    # underneath, then head-pairs 2+3 project + store.
                    tail_mode["on"] = True
                    for th in norm_head:
                        th()
                    for th in tail_fill[:3]:
                        th()
                    for th in norm_tail:
                        th()
                    norm_head, norm_tail = [], []
                    for th in tail_fill[3:]:
                        th()
                    for th in proj_pair_thunks(c4, 1, oacc):
                        th()

    _split_multi_waits(nc, mybir)
    return nc


def _split_multi_waits(nc, mybir):
    """The walrus build in this image rejects instructions carrying more than
    one sem wait ("Too many sync wait commands").  Tile's exit drain carries
    several; peel the extras onto same-engine nops placed just before."""
    for f in nc.m.functions:
        for blk in f.blocks:
            changed = False
            out_list = []
            for inst in blk.instructions:
                si = inst.sync_info
                if si is not None and len(si.on_wait) > 1:
                    waits = list(si.on_wait)
                    for j, w in enumerate(waits[1:]):
                        nop = mybir.InstNoOp(
                            name=f"{inst.name}-wsplit-{j}", ins=[], outs=[],
                            sync_info=mybir.SyncInfo(on_update=[], on_wait=[w]))
                        nop.engine = inst.engine
                        try:
                            nc.register_instruction(nop, overwrite=True)
                        except Exception:
                            pass
                        out_list.append(nop)
                    si.on_wait = waits[:1]
                    inst.sync_info = si
                    changed = True
                out_list.append(inst)
            if changed:
                blk.instructions = out_list


# ------------------------------------------------------------------- host
_cache = {}


def _get_program():
    if "nc" not in _cache:
        _cache["nc"] = build_program()
    return _cache["nc"]


def _pmajor(w):
    """[n*P, m] row-chunked -> partition-major [P, n*m] (contiguous per
    SBUF partition, so the load is one full-rate 2D DMA)."""
    n = w.shape[0] // P
    return np.ascontiguousarray(
        w.reshape(n, P, w.shape[1]).transpose(1, 0, 2).reshape(P, -1))


def make_in_maps(x, W_attn, b_attn, W_proj, b_proj):
    import ml_dtypes

    bf16 = ml_dtypes.bfloat16
    x = np.asarray(x, np.float32)
    W_attn = np.asarray(W_attn, np.float32)
    b_attn = np.asarray(b_attn, np.float32)
    W_proj = np.asarray(W_proj, np.float32)
    CK = C // P
    TI = T // IC
    in_maps = []
    for core in range(NCORES):
        b = core // HGROUPS
        g = core % HGROUPS
        hs = g * DQ
        wq = W_attn[:, hs:hs + DQ]
        wk = W_attn[:, C + hs:C + hs + DQ]
        wv = W_attn[:, 2 * C + hs:2 * C + hs + DQ]
        bq = b_attn[hs:hs + DQ]
        bk = b_attn[C + hs:C + hs + DQ]
        # x[b].T is [C, T]; kernel wants [P, (chunk, ck, i)] contiguous
        xt = np.ascontiguousarray(
            x[b].T.reshape(CK, P, TI, IC).transpose(1, 2, 0, 3).reshape(P, -1))
        in_maps.append({
            "xT": xt.astype(bf16),
            "wq": _pmajor(wq).astype(bf16),
            "wk": _pmajor(wk).astype(bf16),
            "wv": _pmajor(wv).astype(bf16),
            "wpr": _pmajor(W_proj[hs:hs + DQ, :]).astype(bf16),
            "bqk": np.ascontiguousarray(
                np.concatenate([bq, bk]).reshape(2 * (DQ // P), P).T),
        })
    return in_maps


def combine_outputs(results, b_attn, W_proj, b_proj):
    """Sum the two head-group partials per batch; add b_proj plus the
    v-bias term bv @ W_proj (exact: softmax rows sum to 1, so a uniform v
    shift passes straight through the attention)."""
    b_attn = np.asarray(b_attn, np.float32)
    W_proj = np.asarray(W_proj, np.float32)
    b_proj = np.asarray(b_proj, np.float32)
    bv = b_attn[2 * C:3 * C]
    bias_row = bv @ W_proj + b_proj
    y = np.empty((B, T, C), np.float32)
    for b in range(B):
        r0, r1 = results[HGROUPS * b], results[HGROUPS * b + 1]
        y[b] = r0["out"] + r1["out"]
    y += bias_row[None, None, :]
    return y


def kernel(x, W_attn, b_attn, W_proj, b_proj):
    _import_concourse()
    from concourse.bass_utils import run_bass_kernel_spmd

    nc = _get_program()
    in_maps = make_in_maps(x, W_attn, b_attn, W_proj, b_proj)
    res = run_bass_kernel_spmd(nc, in_maps, core_ids=list(range(NCORES)))
    return combine_outputs([res.results[i] for i in range(NCORES)],
                           b_attn, W_proj, b_proj)



# revision 15
# speedup vs baseline: 1.0192x; 1.0192x over previous
"""Causal self-attention (dense transformer attn layer) on 8 Trainium2
NeuronCores.

Sharding: batch x head-group.  Core c handles batch b = c//2 and head-group
g = c%2 (8 of 16 heads).  Each core computes the qkv projection for its head
slice (column-parallel), full causal attention for its 8 heads, and a
row-parallel slice of the output projection.  The host sums the two partial
projection outputs per batch (the "all-reduce") and adds b_proj plus the
v-bias contribution (bv @ W_proj — exact because softmax rows sum to 1).

On-core DRAM layout (per core, T=2048, C=1024, HLOC=8 heads, D=64):
  xT   [C, T]    input slice, transposed on host      (bf16)
  wq/wk [C, 512] W_attn columns, oc-major layout      (bf16)
  wv   [C, 512]  W_attn columns for v                 (bf16)
  wpr  [512, C]  W_proj rows for this group           (bf16)
  bqk  [128, 8]  q/k bias per dout-chunk column       (f32)
  out  [T, C]    partial output                       (bf16)

All big matmuls run in bf16 with fp32 PSUM accumulation.  Weight and x
loads are single rearranged DMAs (HWDGE issue is ~0.6us per dma_start, so
count dominates, not bytes); the k/v/proj weights ride the Activation
HWDGE ring in parallel with the Sync ring.

Pipeline: per 512-token i-chunk c, the attention of chunk c is interleaved
with the qkv projection of chunk c+1 so the PE stays dense.  ALL output
projections are deferred to the final chunk's attention phase, which is
exp-throughput-bound on the Scalar engine and has spare PE capacity.
Per i-chunk:
  1. q(i)^T, k(i)^T = w-stationary @ x(i)^T          -> [d, t] layout
  2. v(i)   = x(i)^T-stationary @ wv -> [t, d] + ones column (for l)
  3. per head-pair, per causal j-block (128 keys):
       S^T[j, i] = k^T(j)-stationary @ q^T(i)   (d=64 row-pair packed, the
                                                 two heads' MMs run
                                                 concurrently in the PE)
       P^T       = exp(S^T/sqrt(d)); diagonal blocks masked via a
                   multiplicative bf16 mask and column-trimmed (both the
                   S and the PV matmuls skip the all-zero column range)
       Yaug^T   += [V|1](j)-stationary @ P^T     (contract j=128, accum)
     Yaug^T row 64 is the softmax denominator l(i).
  4. normalize (DEFERRED into the next head-pair's attention as filler):
     evacuate Yaug^T to SBUF, gather the two l rows into a [2, IC] tile
     (SBUF-SBUF DMA partition shift), 1/l via the DVE fast reciprocal
     (keeps the Scalar engine free for the attention exps), broadcast 1/l
     to 64 partitions via a DRAM bounce, multiply.  The chain DMAs ride
     the GpSimd SWDGE ring so bulk traffic on the Sync ring can't delay
     them (except the final chunk's last pair, which uses Sync for
     latency and is issued ahead of the tail's store traffic).
  5. out(i) = y^T-stationary @ wpr  (contract d=512): chunks 0-2 all run
     as filler inside the final chunk's attention; chunk 2's set is held
     back to keep the PE busy under the tail normalization chain.  The
     final chunk's own projection runs per-head-pair (PSUM-free SBUF
     accumulation) interleaved with its own attention.
"""

import numpy as np

# ---------------------------------------------------------------- constants
B, T, C = 4, 2048, 1024
H, D = 16, 64
NCORES = 8
HGROUPS = NCORES // B          # 2 head groups
HLOC = H // HGROUPS            # 8 heads per core
DQ = HLOC * D                  # 512 head-dims per core
P = 128
IC = 512                       # i-chunk (query) width


def _import_concourse():
    try:
        import concourse.bass  # noqa: F401
    except ImportError:
        import sys

        for p in ("/opt/trn_rl_repo", "/root/.axon_site/_ro/trn_rl_repo"):
            if p not in sys.path:
                sys.path.insert(0, p)
        import concourse.bass  # noqa: F401


def build_program(t=T, c=C, hloc=HLOC, d=D):
    """Build the single-core Bass program (the same program runs SPMD on 8)."""
    _import_concourse()
    import concourse.bass as bass
    import concourse.mybir as mybir
    import concourse.tile as tile

    assert c % P == 0 and t % IC == 0 and hloc % 2 == 0 and d == 64
    dq = hloc * d                  # local q/k/v width
    CK = c // P                    # contraction chunks over channels
    TI = t // IC                   # i-chunks
    JPC = IC // P                  # j-blocks per i-chunk (4)
    DCH = dq // P                  # q/k/y dout chunks
    HP = hloc // 2                 # head pairs
    F32 = mybir.dt.float32
    BF16 = mybir.dt.bfloat16
    EXP = mybir.ActivationFunctionType.Exp
    LN = mybir.ActivationFunctionType.Ln
    SCALE = 1.0 / float(np.sqrt(d))

    # all big inputs are pre-shuffled on the host into partition-major
    # layouts so every load is a fully contiguous 2D DMA on both sides
    # (strided patterns degrade to 1KB packets at ~90ns each on the SDMA
    # engines — ~4x below line rate)
    nc = bass.Bass()
    # xT: chunk 0 is stored as [ih, cc, IC/2] (i-halves, so the first qkv
    # matmuls can start after only wq_oc0 + half of x0 has landed); chunks
    # 1.. keep the [cc, IC] interior.
    xT = nc.declare_dram_parameter("xT", [P, TI * CK * IC], BF16,
                                   isOutput=False)
    # wq/wk are oc-major [oc, cc, P] so a single contiguous 256KB DMA brings
    # in everything the first PSUM group needs.
    wq = nc.declare_dram_parameter("wq", [P, DCH * CK * P], BF16,
                                   isOutput=False)
    wk = nc.declare_dram_parameter("wk", [P, DCH * CK * P], BF16,
                                   isOutput=False)
    wv = nc.declare_dram_parameter("wv", [P, CK * dq], BF16, isOutput=False)
    wpr = nc.declare_dram_parameter("wpr", [P, DCH * c], BF16, isOutput=False)
    bqk = nc.declare_dram_parameter("bqk", [P, 2 * DCH], F32, isOutput=False)
    # partial outputs are stored bf16: the host sums two bf16 partials per
    # batch, which adds ~0.1% rms error (gate is 2e-2) and halves both the
    # store traffic and the end-of-kernel DMA drain.
    out = nc.declare_dram_parameter("out", [t, c], BF16, isOutput=True)

    with tile.TileContext(nc) as tc:
        with (
            nc.allow_low_precision(reason="bf16 matmul inputs, fp32 accum"),
            tc.tile_pool(name="const", bufs=1) as const,
            tc.tile_pool(name="xin", bufs=3) as xin,
            tc.tile_pool(name="qpool", bufs=2) as qpool,
            tc.tile_pool(name="kpool", bufs=TI) as kpool,
            tc.tile_pool(name="vpool", bufs=TI) as vpool,
            tc.tile_pool(name="ypool", bufs=TI) as ypool,
            tc.tile_pool(name="ptp", bufs=10) as ptp,
            tc.tile_pool(name="bcp", bufs=2) as bcp,
            tc.tile_pool(name="ytp", bufs=2) as ytp,
            tc.tile_pool(name="ostage", bufs=2) as ostage,
            tc.tile_pool(name="oacc", bufs=1) as oaccp,
            tc.tile_pool(name="drp", bufs=4, space="DRAM") as drp,
            tc.tile_pool(name="ps_mm", bufs=2, space="PSUM") as ps_mm,
            tc.tile_pool(name="ps_st", bufs=2, space="PSUM") as ps_st,
            tc.tile_pool(name="ps_y", bufs=2, space="PSUM") as ps_y,
        ):
            # ---------------- persistent SBUF state
            wq_sb = const.tile([P, DCH, CK, P], BF16)
            wk_sb = const.tile([P, DCH, CK, P], BF16)
            wv_sb = const.tile([P, CK, dq], BF16)
            wpr_sb = const.tile([P, DCH, c], BF16)
            mask_sb = const.tile([P, JPC, IC], BF16)
            ones_bf = const.tile([P, P], BF16)
            bqk_sb = const.tile([P, 2 * DCH], F32)

            # Startup loads ride one FIFO ring in dependency order: the
            # first qkv PSUM group needs only bqk + wq_oc0 (256KB) + the
            # first i-half of x0 (512KB), so those go first and the first
            # matmul can start at ~2.5us instead of waiting for the full
            # 2MB of wq+x0.
            xt0h0 = xin.tile([P, CK, IC // 2], BF16, tag="x")
            xt0h1 = xin.tile([P, CK, IC // 2], BF16, tag="x")
            nc.sync.dma_start(out=bqk_sb, in_=bqk[:, :])
            nc.sync.dma_start(out=wq_sb[:, 0], in_=wq[:, 0:CK * P])
            nc.sync.dma_start(out=xt0h0, in_=xT[:, 0:CK * (IC // 2)])
            nc.sync.dma_start(out=wq_sb[:, 1:], in_=wq[:, CK * P:])
            nc.sync.dma_start(out=xt0h1,
                              in_=xT[:, CK * (IC // 2):CK * IC])

            # memset can't emit bf16 ones via float32r path; fill f32 and
            # round via DVE copy
            ones_f32 = const.tile([P, P], F32)
            nc.vector.memset(ones_f32, 1.0)
            nc.vector.tensor_copy(out=ones_bf, in_=ones_f32)
            # multiplicative causal masks for the 4 diagonal j-block
            # positions: pattern p is 1 where i_local >= j_local + 128*p
            for pat in range(JPC):
                nc.gpsimd.memset(mask_sb[:, pat, :], 1.0)
                nc.gpsimd.affine_select(
                    out=mask_sb[:, pat, :],
                    in_=mask_sb[:, pat, :],
                    compare_op=mybir.AluOpType.is_ge,
                    fill=0.0,
                    base=-(pat * P),
                    pattern=[[1, IC]],
                    channel_multiplier=-1,
                )

            q_tiles = {}
            k_tiles = {}
            v_tiles = {}
            y_tiles = {}

            def load_x(c4):
                xt = xin.tile([P, CK, IC], BF16, tag="x")
                nc.sync.dma_start(
                    out=xt,
                    in_=xT[:, c4 * CK * IC:(c4 + 1) * CK * IC])
                return xt

            def qkv_thunks(c4, xt_pre=None):
                """One thunk per PSUM accumulation group; called interleaved
                with the previous chunk's attention to keep PE dense.  The
                x chunk is loaded lazily when the first group runs, so its
                DMA doesn't compete with earlier traffic."""
                q_cur = qpool.tile([P, DCH, IC], BF16, tag="q")
                k_cur = kpool.tile([P, DCH, IC], BF16, tag="k")
                v_cur = vpool.tile([P, JPC, hloc, d + 1], BF16, tag="v")
                q_tiles[c4] = q_cur
                k_tiles[c4] = k_cur
                v_tiles[c4] = v_cur
                holder = {}
                if xt_pre is not None:
                    holder["xt"] = xt_pre

                def get_xt():
                    if "xt" not in holder:
                        holder["xt"] = load_x(c4)
                    return holder["xt"]

                thunks = []

                def q_group(oc):
                    xt = get_xt()
                    ps = ps_mm.tile([P, 512], F32, tag="mm")
                    for cc in range(CK):
                        nc.tensor.matmul(
                            ps[:, :IC],
                            lhsT=wq_sb[:, oc, cc, :],
                            rhs=xt[:, cc, :], start=(cc == 0),
                            stop=(cc == CK - 1))
                    nc.vector.tensor_scalar_add(q_cur[:, oc, :], ps[:, :IC],
                                                bqk_sb[:, oc:oc + 1])

                def k_group(oc):
                    xt = get_xt()
                    ps = ps_mm.tile([P, 512], F32, tag="mm")
                    for cc in range(CK):
                        nc.tensor.matmul(
                            ps[:, :IC],
                            lhsT=wk_sb[:, oc, cc, :],
                            rhs=xt[:, cc, :], start=(cc == 0),
                            stop=(cc == CK - 1))
                    nc.vector.tensor_scalar_add(
                        k_cur[:, oc, :], ps[:, :IC],
                        bqk_sb[:, DCH + oc:DCH + oc + 1])

                def v_group(tbl):
                    xt = get_xt()
                    ps = ps_mm.tile([P, 512], F32, tag="mm")
                    for cc in range(CK):
                        nc.tensor.matmul(
                            ps[:, :dq],
                            lhsT=xt[:, cc, tbl * P:(tbl + 1) * P],
                            rhs=wv_sb[:, cc, :], start=(cc == 0),
                            stop=(cc == CK - 1))
                    nc.vector.tensor_copy(
                        out=v_cur[:, tbl, :, 0:d],
                        in_=ps[:, :dq].rearrange("p (h e) -> p h e", h=hloc))
                    # ones column for the softmax-denominator accumulator
                    nc.vector.tensor_copy(
                        out=v_cur[:, tbl, :, d:d + 1],
                        in_=ones_bf[:, 0:hloc][:, :, None])

                for oc in range(DCH):
                    thunks.append(lambda oc=oc: q_group(oc))
                    thunks.append(lambda oc=oc: k_group(oc))
                for tbl in range(JPC):
                    thunks.append(lambda tbl=tbl: v_group(tbl))
                return thunks

            def qkv_chunk0(xh):
                """Chunk 0's qkv, processed in i-halves of 256 so compute can
                begin as soon as wq_oc0 + x0_h0 have landed; group order
                tracks DMA arrival (q first, then k once wk lands, then v)."""
                hw = IC // 2
                q_cur = qpool.tile([P, DCH, IC], BF16, tag="q")
                k_cur = kpool.tile([P, DCH, IC], BF16, tag="k")
                v_cur = vpool.tile([P, JPC, hloc, d + 1], BF16, tag="v")
                q_tiles[0] = q_cur
                k_tiles[0] = k_cur
                v_tiles[0] = v_cur

                def qk_group(w_sb, cur, boff, oc, ih):
                    ps = ps_mm.tile([P, 512], F32, tag="mm")
                    for cc in range(CK):
                        nc.tensor.matmul(
                            ps[:, :hw], lhsT=w_sb[:, oc, cc, :],
                            rhs=xh[ih][:, cc, :], start=(cc == 0),
                            stop=(cc == CK - 1))
                    nc.vector.tensor_scalar_add(
                        cur[:, oc, ih * hw:(ih + 1) * hw], ps[:, :hw],
                        bqk_sb[:, boff + oc:boff + oc + 1])

                def v_group(tbl):
                    ih, tl = divmod(tbl, 2)
                    ps = ps_mm.tile([P, 512], F32, tag="mm")
                    for cc in range(CK):
                        nc.tensor.matmul(
                            ps[:, :dq],
                            lhsT=xh[ih][:, cc, tl * P:(tl + 1) * P],
                            rhs=wv_sb[:, cc, :], start=(cc == 0),
                            stop=(cc == CK - 1))
                    nc.vector.tensor_copy(
                        out=v_cur[:, tbl, :, 0:d],
                        in_=ps[:, :dq].rearrange("p (h e) -> p h e", h=hloc))
                    nc.vector.tensor_copy(
                        out=v_cur[:, tbl, :, d:d + 1],
                        in_=ones_bf[:, 0:hloc][:, :, None])

                for ih in range(2):
                    for oc in range(DCH):
                        qk_group(wq_sb, q_cur, 0, oc, ih)
                for ih in range(2):
                    for oc in range(DCH):
                        qk_group(wk_sb, k_cur, DCH, oc, ih)
                for tbl in range(JPC):
                    v_group(tbl)

            def attention_hp(c4, hp, filler=()):
                """Attention for head-pair hp of chunk c4.  Returns the two
                un-normalized Yaug^T PSUM tiles (normalization is deferred
                into the next head-pair's filler)."""
                filler = list(filler)
                q_cur = q_tiles[c4]
                njb = (c4 + 1) * JPC
                BLK = 2   # j-blocks per S-burst (matches ps_st bufs)
                nblk = (njb + BLK - 1) // BLK
                fill_every = max(1, nblk // len(filler)) if filler else 0
                ya = ps_y.tile([d + 1, IC], F32, tag="y")
                yb = ps_y.tile([d + 1, IC], F32, tag="y")
                blk_i = 0
                for j0 in range(0, njb, BLK):
                    jbs = range(j0, min(j0 + BLK, njb))
                    # burst of S matmuls + exps, then the PV matmuls — the
                    # exp of tile n hides behind the S matmul of tile n+1,
                    # so the PE never micro-idles
                    pts = {}
                    for jb in jbs:
                        kc, jl = jb // JPC, jb % JPC
                        # both heads' S tiles live in one 2-bank PSUM tile,
                        # so one ACT instruction exps the pair
                        st = ps_st.tile([P, 2, IC], F32, tag="st")
                        pt = ptp.tile([P, 2, IC], BF16, tag="pt")
                        pts[jb] = pt
                        diag = jb >= c4 * JPC
                        pat = jb - c4 * JPC if diag else 0
                        w0 = pat * P if diag else 0
                        for hi, po in ((0, 0), (1, 64)):
                            nc.tensor.matmul(
                                st[:, hi, w0:],
                                lhsT=k_tiles[kc][po:po + 64, hp,
                                                 jl * P:(jl + 1) * P],
                                rhs=q_cur[po:po + 64, hp, w0:],
                                start=True, stop=True)
                        nc.scalar.activation(pt[:, :, w0:], st[:, :, w0:],
                                             EXP, scale=SCALE)
                        if diag:
                            nc.vector.tensor_mul(
                                pt[:, :, w0:w0 + P], pt[:, :, w0:w0 + P],
                                mask_sb[:, pat, None,
                                        w0:w0 + P].to_broadcast(
                                            (P, 2, P)))
                    for jb in jbs:
                        diag = jb >= c4 * JPC
                        w0 = (jb - c4 * JPC) * P if diag else 0
                        for hi, po, yps in ((0, 0, ya), (1, 64, yb)):
                            h = 2 * hp + hi
                            # diagonal blocks only contribute columns >= w0
                            # (everything left of the diagonal strip is
                            # masked to zero), so trim the stream
                            nc.tensor.matmul(
                                yps[:, w0:],
                                lhsT=v_tiles[jb // JPC][:, jb % JPC, h, :],
                                rhs=pts[jb][:, hi, w0:],
                                start=(jb == 0), stop=(jb == njb - 1))
                    blk_i += 1
                    if filler and fill_every and blk_i % fill_every == 0:
                        filler.pop(0)()
                for th in filler:
                    th()
                return ya, yb

            def normalize_thunks(c4, hp, ya, yb, tail=False):
                """Deferred normalization of head-pair hp.  Returns
                (t0, t1, t2): t0 runs inline right after the attention
                (evacuates PSUM so the next pair's PV can start); t1/t2 are
                pushed into the next head-pair's filler, t2 late enough
                that the broadcast DMA has landed.  `tail` (final pair of
                the final chunk) splits the evacuation across DVE+ACT to
                shorten the end-of-kernel chain."""
                y_cur = y_tiles[c4]
                state = {}

                def t0():
                    # evacuate both Yaug^T tiles into one [P, 2, IC] SBUF
                    # staging tile; the PSUM banks free after these copies
                    ycp = bcp.tile([P, 2, IC], F32, tag="ycp")
                    nc.vector.tensor_copy(out=ycp[0:d + 1, 0, :],
                                          in_=ya[0:d + 1, :])
                    if tail:
                        # ACT is idle once the last exp retires; run the
                        # second copy there so the two evacuations overlap
                        nc.scalar.copy(out=ycp[0:d + 1, 1, :],
                                       in_=yb[0:d + 1, :])
                    else:
                        nc.vector.tensor_copy(out=ycp[0:d + 1, 1, :],
                                              in_=yb[0:d + 1, :])
                        # gather the two l rows (partition 64, head slots
                        # 0/1) into partitions 0..1 (the DMA does the
                        # partition shift)
                        lg = bcp.tile([2, IC], F32, tag="lg")
                        nc.sync.dma_start(out=lg[0:2, :],
                                          in_=ycp[d:d + 1, :, :])
                        state["lg"] = lg
                    state["ycp"] = ycp

                def t1():
                    # 1/l = exp(-ln(l)) on ScalarE, batched over both heads
                    # (this walrus build rejects the custom-DVE fast
                    # reciprocal: "ISA wrong length")
                    bcs = bcp.tile([d, 2, IC], F32, tag="bcs")
                    ycp = state["ycp"]
                    if tail:
                        # latency-critical end-of-kernel chain: keep 1/l at
                        # partition 64, broadcast with DVE stream_shuffles
                        # instead of DMA hops, and finish head A entirely
                        # before touching head B so its multiply (which
                        # gates the final projection) starts earliest.
                        lnl = bcp.tile([P, 2, IC], F32, tag="lnl")
                        nc.scalar.activation(lnl[d:d + 1, :, :],
                                             ycp[d:d + 1, :, :], LN)
                        rinv = bcp.tile([P, 2, IC], F32, tag="rinvt")
                        nc.scalar.activation(rinv[d:d + 1, :, :],
                                             lnl[d:d + 1, :, :], EXP,
                                             scale=-1.0)
                        for hi in range(2):
                            nc.vector.stream_shuffle(
                                out=bcs[0:32, hi:hi + 1, :],
                                in_=rinv[64:96, hi:hi + 1, :],
                                mask=[0] * 32)
                            nc.vector.stream_shuffle(
                                out=bcs[32:64, hi:hi + 1, :],
                                in_=rinv[64:96, hi:hi + 1, :],
                                mask=[0] * 32)
                            if hi == 0:
                                nc.vector.tensor_mul(y_cur[0:d, hp, :],
                                                     ycp[0:d, 0, :],
                                                     bcs[:, 0, :])
                    else:
                        lnl = bcp.tile([2, IC], F32, tag="lnl")
                        nc.scalar.activation(lnl, state["lg"], LN)
                        rinv = bcp.tile([2, IC], F32, tag="rinv")
                        nc.scalar.activation(rinv, lnl, EXP, scale=-1.0)
                        rd = drp.tile([2, IC], F32, tag="rd")
                        nc.sync.dma_start(out=rd, in_=rinv)
                        # partition-broadcast 1/l by bouncing through DRAM
                        # (DRAM DMA sources may repeat across partitions;
                        # SBUF sources may not)
                        nc.sync.dma_start(
                            out=bcs,
                            in_=rd[None, :, :].to_broadcast((d, 2, IC)))
                    state["bcs"] = bcs

                def t2():
                    ycp = state["ycp"]
                    bcs = state["bcs"]
                    if not tail:
                        nc.vector.tensor_mul(y_cur[0:d, hp, :],
                                             ycp[0:d, 0, :], bcs[:, 0, :])
                    yt = ytp.tile([P, IC], BF16, tag="yt")
                    nc.vector.tensor_mul(yt[0:d, :],
                                         ycp[0:d, 1, :], bcs[:, 1, :])
                    # shift head B to partitions 64..127
                    if tail:
                        nc.vector.stream_shuffle(
                            out=y_cur[64:96, hp, :], in_=yt[0:32, :],
                            mask=list(range(32)))
                        nc.vector.stream_shuffle(
                            out=y_cur[96:P, hp, :], in_=yt[32:64, :],
                            mask=list(range(32)))
                    else:
                        nc.sync.dma_start(out=y_cur[64:P, hp, :],
                                          in_=yt[0:d, :])

                return t0, t1, t2

            def proj_pair_thunks(c4, pair, oacc):
                """Projection contribution of head-pairs (2*pair, 2*pair+1)
                for the final chunk: two PSUM-accumulated matmuls per output
                tile (denser PE than one-matmul-per-DVE-add), then one DVE
                copy/add into the SBUF accumulator.  Pair 1 adds into a bf16
                staging row and stores the full 1024-wide output row-block
                with a single DMA."""
                y_cur = y_tiles[c4]

                def grp(tbl):
                    obf = None
                    if pair == 1:
                        obf = ostage.tile([P, c], BF16, tag="ost",
                                          name=f"obf_{tbl}")
                    for oh in range(c // 512):
                        ps = ps_mm.tile([P, 512], F32, tag="mm")
                        for k, hp in enumerate((2 * pair, 2 * pair + 1)):
                            nc.tensor.matmul(
                                ps,
                                lhsT=y_cur[:, hp, tbl * P:(tbl + 1) * P],
                                rhs=wpr_sb[:, hp, oh * 512:(oh + 1) * 512],
                                start=(k == 0), stop=(k == 1))
                        if pair == 0:
                            nc.vector.tensor_copy(out=oacc[:, tbl, oh, :],
                                                  in_=ps)
                        else:
                            nc.vector.tensor_add(
                                obf[:, oh * 512:(oh + 1) * 512],
                                oacc[:, tbl, oh, :], ps)
                    if pair == 1:
                        tb = c4 * JPC + tbl
                        nc.sync.dma_start(out=out[tb * P:(tb + 1) * P, :],
                                          in_=obf)

                return [lambda tbl=tbl: grp(tbl) for tbl in range(JPC)]

            # during the tail the DVE is saturated by the normalization
            # chain; evacuating the held-back projection groups on the
            # (then-idle) Scalar engine keeps ps_mm turning over so the PE
            # never starves behind the DVE queue
            tail_mode = {"on": False}

            def proj_thunks(c4):
                def grp(tbl):
                    y_cur = y_tiles[c4]
                    tb = c4 * JPC + tbl
                    ost = ostage.tile([P, c], BF16, tag="ost")
                    for oh in range(c // 512):
                        ps = ps_mm.tile([P, 512], F32, tag="mm")
                        for dc in range(DCH):
                            nc.tensor.matmul(
                                ps,
                                lhsT=y_cur[:, dc, tbl * P:(tbl + 1) * P],
                                rhs=wpr_sb[:, dc, oh * 512:(oh + 1) * 512],
                                start=(dc == 0), stop=(dc == DCH - 1))
                        if tail_mode["on"]:
                            nc.scalar.copy(out=ost[:, oh * 512:(oh + 1) * 512],
                                           in_=ps)
                        else:
                            nc.vector.tensor_copy(
                                out=ost[:, oh * 512:(oh + 1) * 512], in_=ps)
                    nc.sync.dma_start(out=out[tb * P:(tb + 1) * P, :],
                                      in_=ost)

                return [lambda tbl=tbl: grp(tbl) for tbl in range(JPC)]

            # -------------- software pipeline over i-chunks
            # later-needed weights go on the SAME Sync ring AFTER x0: the
            # ring is FIFO, so wq+x0 (which gate the first matmuls) get the
            # full SDMA bandwidth first.  (A second ring halves their
            # bandwidth — the 16 SDMA engines round-robin between rings —
            # and the Tile scheduler reorders cross-engine issue, so a
            # dummy-dependency gate doesn't work.)
            nc.sync.dma_start(out=wk_sb, in_=wk[:, :])
            nc.sync.dma_start(out=wv_sb, in_=wv[:, :])
            nc.sync.dma_start(out=wpr_sb, in_=wpr[:, :])
            qkv_chunk0([xt0h0, xt0h1])
            # projections of chunks 0-2 are ALL deferred to the final
            # chunk's attention phase (exp-bound on ScalarE, spare PE);
            # chunk 2's set is held back as tail filler.
            proj_backlog = []
            norm_head = []
            norm_tail = []
            # t2 of pair p (the deferred 1/l multiplies) runs right after
            # pair p+1's PSUM evacuation instead of inside p+1's filler: by
            # then its broadcast DMA has surely landed, so it can never
            # head-of-line-block the mask-muls in the DVE queue (the ~3us
            # PE stall the baseline hit at each chunk boundary).
            prev_t2 = None
            for c4 in range(TI):
                last = c4 + 1 >= TI
                pend = []
                if not last:
                    pend += qkv_thunks(c4 + 1)
                    oacc = None
                    tail_fill = []
                else:
                    tail_fill = proj_backlog.pop()      # chunk 2's proj set
                    while proj_backlog:
                        pend += proj_backlog.pop(0)     # chunks 0, 1
                    oacc = oaccp.tile([P, JPC, c // 512, 512], F32,
                                      name="oacc")
                y_tiles[c4] = ypool.tile([P, DCH, IC], BF16, tag="ych",
                                         name=f"ych_{c4}")
                per_hp = (len(pend) + HP - 1) // HP if pend else 0
                carry_proj = []
                for hp in range(HP):
                    fill = (norm_head
                            + pend[hp * per_hp:(hp + 1) * per_hp]
                            + norm_tail
                            + carry_proj)
                    carry_proj = []
                    norm_tail = []
                    ya, yb = attention_hp(c4, hp, filler=fill)
                    t0, t1, t2 = normalize_thunks(c4, hp, ya, yb,
                                                  tail=(last and hp == HP - 1))
                    t0()
                    if prev_t2 is not None:
                        prev_t2()
                        prev_t2 = None
                    if last and hp == 1:
                        # hp1's t2 must pop inside hp2's filler BEFORE the
                        # pair-0 projection (which reads y_cur[:, 0:2]) —
                        # deferring it past hp2's attention would deadlock
                        # the PE queue behind the DVE queue.
                        norm_tail = [t2]
                        carry_proj = proj_pair_thunks(c4, 0, oacc)
                    else:
                        prev_t2 = t2
                    norm_head = [t1]
                if not last:
                    proj_backlog.append(proj_thunks(c4))
                else:
                    # tail: the last head-pair's normalization chain runs
                    # with chunk 2's projection keeping the PE warm
                    # underneath, then head-pairs 2+3 project + store.
                    tail_mode["on"] = True
                    for th in norm_head:
                        th()
                    for th in tail_fill[:2]:
                        th()
                    prev_t2()
                    prev_t2 = None
                    norm_head, norm_tail = [], []
                    for th in tail_fill[2:]:
                        th()
                    for th in proj_pair_thunks(c4, 1, oacc):
                        th()

    _split_multi_waits(nc, mybir)
    return nc


def _split_multi_waits(nc, mybir):
    """The walrus build in this image rejects instructions carrying more than
    one sem wait ("Too many sync wait commands").  Tile's exit drain carries
    several; peel the extras onto same-engine nops placed just before."""
    for f in nc.m.functions:
        for blk in f.blocks:
            changed = False
            out_list = []
            for inst in blk.instructions:
                si = inst.sync_info
                if si is not None and len(si.on_wait) > 1:
                    waits = list(si.on_wait)
                    for j, w in enumerate(waits[1:]):
                        nop = mybir.InstNoOp(
                            name=f"{inst.name}-wsplit-{j}", ins=[], outs=[],
                            sync_info=mybir.SyncInfo(on_update=[], on_wait=[w]))
                        nop.engine = inst.engine
                        try:
                            nc.register_instruction(nop, overwrite=True)
                        except Exception:
                            pass
                        out_list.append(nop)
                    si.on_wait = waits[:1]
                    inst.sync_info = si
                    changed = True
                out_list.append(inst)
            if changed:
                blk.instructions = out_list


# ------------------------------------------------------------------- host
_cache = {}


def _get_program():
    if "nc" not in _cache:
        _cache["nc"] = build_program()
    return _cache["nc"]


def _pmajor(w):
    """[n*P, m] row-chunked -> partition-major [P, n*m] (contiguous per
    SBUF partition, so the load is one full-rate 2D DMA)."""
    n = w.shape[0] // P
    return np.ascontiguousarray(
        w.reshape(n, P, w.shape[1]).transpose(1, 0, 2).reshape(P, -1))


def _pmajor_oc(w):
    """[CK*P, DCH*P] weight -> [P, DCH, CK, P] oc-major partition layout
    (one contiguous DMA per output-channel group)."""
    ck = w.shape[0] // P
    dch = w.shape[1] // P
    return np.ascontiguousarray(
        w.reshape(ck, P, dch, P).transpose(1, 2, 0, 3).reshape(P, -1))


def make_in_maps(x, W_attn, b_attn, W_proj, b_proj):
    import ml_dtypes

    bf16 = ml_dtypes.bfloat16
    x = np.asarray(x, np.float32)
    W_attn = np.asarray(W_attn, np.float32)
    b_attn = np.asarray(b_attn, np.float32)
    W_proj = np.asarray(W_proj, np.float32)
    CK = C // P
    TI = T // IC
    in_maps = []
    for core in range(NCORES):
        b = core // HGROUPS
        g = core % HGROUPS
        hs = g * DQ
        wq = W_attn[:, hs:hs + DQ]
        wk = W_attn[:, C + hs:C + hs + DQ]
        wv = W_attn[:, 2 * C + hs:2 * C + hs + DQ]
        bq = b_attn[hs:hs + DQ]
        bk = b_attn[C + hs:C + hs + DQ]
        # x[b].T is [C, T]; kernel wants [P, (chunk, ck, i)] contiguous,
        # except chunk 0 whose interior is [P, (ihalf, ck, i/2)] so the
        # kernel can start on half of x0.
        arr = x[b].T.reshape(CK, P, TI, IC)
        x0 = np.ascontiguousarray(
            arr[:, :, 0, :].reshape(CK, P, 2, IC // 2)
            .transpose(1, 2, 0, 3).reshape(P, -1))
        xrest = np.ascontiguousarray(
            arr[:, :, 1:, :].transpose(1, 2, 0, 3).reshape(P, -1))
        xt = np.concatenate([x0, xrest], axis=1)
        in_maps.append({
            "xT": xt.astype(bf16),
            "wq": _pmajor_oc(wq).astype(bf16),
            "wk": _pmajor_oc(wk).astype(bf16),
            "wv": _pmajor(wv).astype(bf16),
            "wpr": _pmajor(W_proj[hs:hs + DQ, :]).astype(bf16),
            "bqk": np.ascontiguousarray(
                np.concatenate([bq, bk]).reshape(2 * (DQ // P), P).T),
        })
    return in_maps


def combine_outputs(results, b_attn, W_proj, b_proj):
    """Sum the two head-group partials per batch; add b_proj plus the
    v-bias term bv @ W_proj (exact: softmax rows sum to 1, so a uniform v
    shift passes straight through the attention)."""
    b_attn = np.asarray(b_attn, np.float32)
    W_proj = np.asarray(W_proj, np.float32)
    b_proj = np.asarray(b_proj, np.float32)
    bv = b_attn[2 * C:3 * C]
    bias_row = bv @ W_proj + b_proj
    y = np.empty((B, T, C), np.float32)
    for b in range(B):
        r0, r1 = results[HGROUPS * b], results[HGROUPS * b + 1]
        y[b] = (np.asarray(r0["out"], np.float32)
                + np.asarray(r1["out"], np.float32))
    y += bias_row[None, None, :]
    return y


def kernel(x, W_attn, b_attn, W_proj, b_proj):
    _import_concourse()
    from concourse.bass_utils import run_bass_kernel_spmd

    nc = _get_program()
    in_maps = make_in_maps(x, W_attn, b_attn, W_proj, b_proj)
    res = run_bass_kernel_spmd(nc, in_maps, core_ids=list(range(NCORES)))
    return combine_outputs([res.results[i] for i in range(NCORES)],
                           b_attn, W_proj, b_proj)

